# revision 1
# baseline (speedup 1.0000x reference)
"""AgreementRouting (CapsNet dynamic routing) Trainium2 kernel.

Data-parallel over batch B=128 across 8 cores (B_local=16 per core).

Per core, u lives in SBUF as fp16 in an "H layout":
  partition p = b_loc*16 + d   (b_loc in [0,8), d in [0,16))
  free       = (j in [0,10), h in [0,2), i in [0,1152))
local batch index beta = h*8 + b_loc.

Key structural idea: the routing iteration keeps the *unnormalized* s
vector as the PE stationary (s_fat, block-diagonal [128,80] per (j,h));
the squash scale f = sq/((1+sq)sqrt(sq+EPS)) is folded into the logit
update as a per-partition scalar on the [80]-partition side:
    bb += f[b,j] * (sum_d s[b,j,d] * u[(b,d), i])
so the squash never sits on the serial chain, and next-iteration W1
matmuls are gated only by the previous iteration's weighted-sum columns.

Pipeline per iteration (i-chunked at 512 for PSUM-bank granularity):
  W1: PE accumulating matmuls a_c = sum_j s_fat[j].T @ u16[j]   (fp16)
  bb += f80 * a_c                   (DVE scalar_tensor_tensor, in-place)
  e = exp(bb)                       (ACT, shared Exp/Ln table)
  Z-replicated = zselrep.T @ e      (PE, f32, [80,*] pre-replicated)
  lzr = ln(Z)                       (ACT)
  tm = bb - lzr                     (DVE)
  c16 = exp(tm)                     (ACT, fp16)
  W3: c_rep = sel_j.T @ c16         (PE fp16 selector expand to (b,d))
  W4: s_col = sum_i u16*c_rep       (DVE STT fused accumulate; a few j
                                     routed via ACT-copy + GpSimd STT)
  s_fat/f80 rebuilt incrementally as s columns complete.

Numerics vs the fp32 oracle: absmax/scale ~ 5e-4 (verified in numpy,
CoreSim, and on hardware).
"""

import os
import sys

import numpy as np

for _p in ("/opt/trn_rl_repo", "/opt/trn_rl_repo/concourse"):
    if _p not in sys.path and os.path.isdir(_p):
        sys.path.insert(0, _p)

B, IC, OC, D = 128, 1152, 10, 16
NCORES = 8
BL = B // NCORES          # 16 local batch
H = 2                     # halves of local batch
BLOC = BL // H            # 8
NI = IC                   # 1152
EPS = 1e-8
NITER = 3
CHUNKS = [(0, 512), (512, 1024), (1024, 1152)]
NGP_J = int(os.environ.get("K_NGP_J", "0"))  # j's routed to GpSimd per h
NMIX_J = int(os.environ.get("K_NMIX_J", "0"))  # j's routed via ACT-assist
TIME_REPS = int(os.environ.get("K_TIME_REPS", "1"))  # whole-program reps

_PROG_CACHE = {}


def _build_consts():
    """Host-side constant selector matrices."""
    # sel16[(j2,b), j, (bl,d)] = 1 if j2==j and b==bl  -> [80, 10, 128] fp16
    sel = np.zeros((OC * BLOC, OC, BLOC * D), np.float16)
    for j in range(OC):
        for b in range(BLOC):
            sel[j * BLOC + b, j, b * D:(b + 1) * D] = 1.0
    # zselrep[(j,b), (j2,b2)] = 1 if b==b2               -> [80, 80] f32
    zselrep = np.zeros((OC * BLOC, OC * BLOC), np.float32)
    for j in range(OC):
        for b in range(BLOC):
            for j2 in range(OC):
                zselrep[j * BLOC + b, j2 * BLOC + b] = 1.0
    # base8[(b,d), b2] = 1 if b==b2                      -> [128, 8] f32
    base8 = np.zeros((BLOC * D, BLOC), np.float32)
    for b in range(BLOC):
        base8[b * D:(b + 1) * D, b] = 1.0
    dsel = base8
    drep = np.ascontiguousarray(base8.T)
    return dict(sel16=sel, zselrep=zselrep, base8=base8,
                dsel=dsel, drep=drep)


def _build_program(general_b):
    import concourse.bacc as bacc
    import concourse.mybir as mybir
    import concourse.tile as tile

    dt = mybir.dt
    AF = mybir.ActivationFunctionType
    ALU = mybir.AluOpType

    # Force a single shared ACT table (Exp+Ln+Copy+Identity in one set) so
    # the table-load pass emits one load instead of thrashing per func.
    from concourse import hw_specs as _hws
    _orig_tabs = _hws.get_activation_tables
    _keep = "natural_log_exp_and_others"

    def _patched_tabs(arch, __orig=_orig_tabs, __keep=_keep):
        tabs = __orig(arch)
        return {n: (s if n == __keep else set()) for n, s in tabs.items()}

    bacc.get_activation_tables = _patched_tabs

    nc = bacc.Bacc("TRN2", target_bir_lowering=False, debug=False)

    # ---- DRAM I/O ----
    u16_d = nc.dram_tensor("u16", [128, OC, H, NI], dt.float16,
                           kind="ExternalInput").ap()
    sel_d = nc.dram_tensor("sel16", [OC * BLOC, OC, BLOC * D], dt.float16,
                           kind="ExternalInput").ap()
    zselrep_d = nc.dram_tensor("zselrep", [OC * BLOC, OC * BLOC], dt.float32,
                               kind="ExternalInput").ap()
    base8_d = nc.dram_tensor("base8", [BLOC * D, BLOC], dt.float32,
                             kind="ExternalInput").ap()
    dsel_d = nc.dram_tensor("dsel", [BLOC * D, BLOC], dt.float32,
                            kind="ExternalInput").ap()
    drep_d = nc.dram_tensor("drep", [BLOC, BLOC * D], dt.float32,
                            kind="ExternalInput").ap()
    if general_b:
        c0_d = nc.dram_tensor("c0rep", [128, OC, NI], dt.float16,
                              kind="ExternalInput").ap()
        bb0_d = nc.dram_tensor("bb0", [OC * BLOC, NI], dt.float32,
                               kind="ExternalInput").ap()
    out_d = nc.dram_tensor("vout", [128, 2 * OC], dt.float32,
                           kind="ExternalOutput").ap()

    # ---- static SBUF ----
    def sb(name, shape, dtype):
        return nc.alloc_sbuf_tensor(name, list(shape), dtype).ap()

    u16 = sb("u16_sb", [128, OC * H * NI], dt.float16)       # 46KB/part
    sel_sb = sb("sel_sb", [OC * BLOC, OC * BLOC * D], dt.float16)
    zselrep_sb = sb("zselrep_sb", [OC * BLOC, OC * BLOC], dt.float32)
    base8_sb = sb("base8_sb", [BLOC * D, BLOC], dt.float32)
    dsel_sb = sb("dsel_sb", [BLOC * D, BLOC], dt.float32)
    drep_sb = sb("drep_sb", [BLOC, BLOC * D], dt.float32)
    bb = [sb(f"bbsb{h}", [OC * BLOC, NI], dt.float32) for h in range(H)]
    sfat = [[sb(f"sfat{j}_{h}", [128, OC * BLOC], dt.float16)
             for h in range(H)] for j in range(OC)]
    f80 = [sb(f"f80_{h}", [OC * BLOC, 1], dt.float32) for h in range(H)]
    s_sb = sb("s_sb", [128, H * OC], dt.float32)
    # mini-squash scratch (per h)
    ssqh = [sb(f"ssqh{h}", [128, OC], dt.float32) for h in range(H)]
    sepsh = [sb(f"sepsh{h}", [BLOC, OC], dt.float32) for h in range(H)]
    lnxh = [sb(f"lnxh{h}", [BLOC, OC], dt.float32) for h in range(H)]
    rh = [sb(f"rh{h}", [BLOC, OC], dt.float32) for h in range(H)]
    t1h = [sb(f"t1h{h}", [BLOC, OC], dt.float32) for h in range(H)]
    denh = [sb(f"denh{h}", [BLOC, OC], dt.float32) for h in range(H)]
    rech = [sb(f"rech{h}", [BLOC, OC], dt.float32) for h in range(H)]
    fh = [sb(f"fh{h}", [BLOC, OC], dt.float32) for h in range(H)]
    # final squash scratch
    ssq_sb = sb("ssq_sb", [128, H * OC], dt.float32)
    seps_sb = sb("seps_sb", [BLOC, H * OC], dt.float32)
    lnx_sb = sb("lnx_sb", [BLOC, H * OC], dt.float32)
    r_sb = sb("r_sb", [BLOC, H * OC], dt.float32)
    t1_sb = sb("t1_sb", [BLOC, H * OC], dt.float32)
    den_sb = sb("den_sb", [BLOC, H * OC], dt.float32)
    rec_sb = sb("rec_sb", [BLOC, H * OC], dt.float32)
    f_sb = sb("f_sb", [BLOC, H * OC], dt.float32)
    v_sb = sb("v_sb", [128, H * OC], dt.float32)

    def uview(j, h):
        off = (j * H + h) * NI
        return u16[:, off:off + NI]

    def selview(j):
        return sel_sb[:, j * BLOC * D:(j + 1) * BLOC * D]

    with tile.TileContext(nc) as tc:
        from contextlib import ExitStack
        with ExitStack() as ctx:
            psA = ctx.enter_context(
                tc.tile_pool(name="psA", bufs=2, space="PSUM"))
            psB = ctx.enter_context(
                tc.tile_pool(name="psB", bufs=2, space="PSUM"))
            sc = ctx.enter_context(tc.tile_pool(name="sc", bufs=int(os.environ.get("K_SCBUFS", "3"))))
            ec = ctx.enter_context(tc.tile_pool(name="ec", bufs=int(os.environ.get("K_ECBUFS", "4"))))

            for _rep in range(TIME_REPS):
                # ---- loads ----
                for j in range(OC):
                    nc.sync.dma_start(
                        u16[:, j * H * NI:(j + 1) * H * NI], u16_d[:, j, :, :])
                nc.sync.dma_start(sel_sb[:], sel_d.rearrange("p j m -> p (j m)"))
                nc.sync.dma_start(zselrep_sb[:], zselrep_d)
                nc.sync.dma_start(base8_sb[:], base8_d)
                nc.sync.dma_start(dsel_sb[:], dsel_d)
                nc.sync.dma_start(drep_sb[:], drep_d)

                # ---- init bb and sfat ----
                for h in range(H):
                    if general_b:
                        nc.sync.dma_start(bb[h][:], bb0_d)
                    else:
                        nc.gpsimd.memset(bb[h][:], 0.0)
                for j in range(OC):
                    for h in range(H):
                        nc.gpsimd.memset(sfat[j][h][:], 0.0)

                def build_sfat(j, h):
                    col = 2 * j + h
                    nc.vector.tensor_scalar(
                        out=sfat[j][h][:, j * BLOC:(j + 1) * BLOC],
                        in0=base8_sb[:],
                        scalar1=s_sb[:, col:col + 1],
                        scalar2=None,
                        op0=ALU.mult)

                def mini_squash(h):
                    """f80[h] <- squash scale from s_sb columns of parity h."""
                    s_h = s_sb[:, h::2]  # [128, OC] strided view
                    nc.vector.tensor_tensor(ssqh[h][:], s_h, s_h, op=ALU.mult)
                    sq_ps = psB.tile([BLOC, OC], dt.float32, tag="bank",
                                     name="sq_ps")
                    nc.tensor.matmul(sq_ps[:], dsel_sb[:], ssqh[h][:],
                                     start=True, stop=True)
                    nc.vector.tensor_scalar_add(sepsh[h][:], sq_ps[:], EPS)
                    nc.scalar.activation(lnxh[h][:], sepsh[h][:], AF.Ln)
                    nc.scalar.activation(rh[h][:], lnxh[h][:], AF.Exp, scale=0.5)
                    nc.vector.tensor_scalar_add(t1h[h][:], sq_ps[:], 1.0)
                    nc.vector.tensor_tensor(denh[h][:], t1h[h][:], rh[h][:],
                                            op=ALU.mult)
                    nc.vector.reciprocal(rech[h][:], denh[h][:])
                    nc.vector.tensor_tensor(fh[h][:], sq_ps[:],
                                            rech[h][:], op=ALU.mult)
                    for j in range(OC):
                        nc.sync.dma_start(
                            f80[h][j * BLOC:(j + 1) * BLOC, 0:1],
                            fh[h][:, j:j + 1])

                # ---- init s0 ----
                if general_b:
                    c0_sb = sc.tile([128, OC * NI], dt.float16, tag="c0",
                                    name="c0_sb", bufs=1)
                    nc.sync.dma_start(c0_sb[:], c0_d)
                    for j in range(OC):
                        for h in range(H):
                            col = 2 * j + h
                            scr = sc.tile([128, NI], dt.float16, tag="scr",
                                          name="scr")
                            nc.vector.scalar_tensor_tensor(
                                out=scr[:], in0=uview(j, h), scalar=1.0,
                                in1=c0_sb[:, j * NI:(j + 1) * NI],
                                op0=ALU.mult, op1=ALU.mult,
                                accum_out=s_sb[:, col:col + 1])
                else:
                    AX = mybir.AxisListType
                    for j in range(OC):
                        if j < 4 or j >= 8:
                            # DVE: reduce both h at once
                            off = j * H * NI
                            nc.vector.reduce_sum(
                                s_sb[:, 2 * j:2 * j + 2],
                                u16[:, off:off + H * NI].rearrange(
                                    "p (h i) -> p h i", h=H),
                                axis=AX.X)
                        else:
                            for h in range(H):
                                col = 2 * j + h
                                scr = sc.tile([128, NI], dt.float16, tag="scr",
                                              name="scr")
                                nc.scalar.activation(
                                    scr[:], uview(j, h), AF.Identity,
                                    accum_out=s_sb[:, col:col + 1])
                if not general_b:
                    nc.vector.tensor_scalar_mul(s_sb[:], s_sb[:], 1.0 / OC)
                for j in range(OC):
                    for h in range(H):
                        build_sfat(j, h)
                for h in range(H):
                    mini_squash(h)

                # ---- routing iterations ----
                for it in range(NITER):
                    last = it == NITER - 1
                    # stage-major emission: each engine streams all 6 chunks
                    # (2h x 3c) back-to-back, so cross-engine latency amortizes.
                    hc = [(h, ci) for h in range(H) for ci in range(len(CHUNKS))]
                    a_t = {}
                    for (h, ci) in hc:
                        c0, c1 = CHUNKS[ci]
                        w = c1 - c0
                        a_c = psB.tile([OC * BLOC, 512], dt.float32,
                                       tag="bank", name="a_c")
                        for j in range(OC):
                            nc.tensor.matmul(
                                a_c[:, :w], sfat[j][h],
                                uview(j, h)[:, c0:c1],
                                start=(j == 0), stop=(j == OC - 1))
                        a_t[(h, ci)] = a_c
                    for (h, ci) in hc:
                        c0, c1 = CHUNKS[ci]
                        nc.vector.scalar_tensor_tensor(
                            out=bb[h][:, c0:c1], in0=a_t[(h, ci)][:, :c1 - c0],
                            scalar=f80[h][:, 0:1], in1=bb[h][:, c0:c1],
                            op0=ALU.mult, op1=ALU.add)
                    e_t = {}
                    for (h, ci) in hc:
                        c0, c1 = CHUNKS[ci]
                        e_c = ec.tile([OC * BLOC, 512], dt.float32,
                                      tag="e", name="e_c")
                        nc.scalar.activation(e_c[:, :c1 - c0], bb[h][:, c0:c1],
                                             AF.Exp)
                        e_t[(h, ci)] = e_c
                    z_t = {}
                    for (h, ci) in hc:
                        c0, c1 = CHUNKS[ci]
                        z_c = psA.tile([OC * BLOC, 512], dt.float32,
                                       tag="big", name="z_c")
                        nc.tensor.matmul(z_c[:, :c1 - c0], zselrep_sb[:],
                                         e_t[(h, ci)][:, :c1 - c0],
                                         start=True, stop=True)
                        z_t[(h, ci)] = z_c
                    lzr_t = {}
                    for (h, ci) in hc:
                        c0, c1 = CHUNKS[ci]
                        lzr_c = ec.tile([OC * BLOC, 512], dt.float32,
                                        tag="lzr", name="lzr_c")
                        nc.scalar.activation(lzr_c[:, :c1 - c0],
                                             z_t[(h, ci)][:, :c1 - c0], AF.Ln)
                        lzr_t[(h, ci)] = lzr_c
                    tm_t = {}
                    for (h, ci) in hc:
                        c0, c1 = CHUNKS[ci]
                        tm_c = ec.tile([OC * BLOC, 512], dt.float32,
                                       tag="tm", name="tm_c")
                        nc.vector.tensor_tensor(tm_c[:, :c1 - c0],
                                                bb[h][:, c0:c1],
                                                lzr_t[(h, ci)][:, :c1 - c0],
                                                op=ALU.subtract)
                        tm_t[(h, ci)] = tm_c
                    c16 = {}
                    for h in range(H):
                        c16[h] = ec.tile([OC * BLOC, NI], dt.float16,
                                         tag="c16", name="c16")
                    for (h, ci) in hc:
                        c0, c1 = CHUNKS[ci]
                        nc.scalar.activation(c16[h][:, c0:c1],
                                             tm_t[(h, ci)][:, :c1 - c0], AF.Exp)
                    for h in range(H):
                        for j in range(OC):
                            col = 2 * j + h
                            cr_ps = psA.tile([128, NI], dt.float32, tag="big",
                                             name="cr_ps")
                            for (c0, c1) in CHUNKS:
                                nc.tensor.matmul(cr_ps[:, c0:c1], selview(j),
                                                 c16[h][:, c0:c1],
                                                 start=True, stop=True)
                            if j < NMIX_J:
                                # ACT copy psum->fp16, DVE TT at 2x, ACT
                                # fused accumulate-reduce
                                crs = sc.tile([128, NI], dt.float16, tag="crs",
                                              name="crs")
                                nc.scalar.copy(crs[:], cr_ps[:])
                                w16 = sc.tile([128, NI], dt.float16,
                                              tag="w16", name="w16")
                                nc.vector.tensor_tensor(w16[:], uview(j, h),
                                                        crs[:], op=ALU.mult)
                                scr = sc.tile([128, NI], dt.float16,
                                              tag="scrg", name="scr")
                                nc.scalar.activation(
                                    scr[:], w16[:], AF.Identity,
                                    accum_out=s_sb[:, col:col + 1])
                            else:
                                scr = sc.tile([128, NI], dt.float16, tag="scr",
                                              name="scr")
                                nc.vector.scalar_tensor_tensor(
                                    out=scr[:], in0=uview(j, h), scalar=1.0,
                                    in1=cr_ps[:],
                                    op0=ALU.mult, op1=ALU.mult,
                                    accum_out=s_sb[:, col:col + 1])
                            if not last:
                                build_sfat(j, h)
                        if not last:
                            mini_squash(h)

                # ---- final squash -> v ----
                nc.vector.tensor_tensor(ssq_sb[:], s_sb[:], s_sb[:], op=ALU.mult)
                sq_ps = psB.tile([BLOC, H * OC], dt.float32, tag="bank",
                                 name="sq_ps")
                nc.tensor.matmul(sq_ps[:], dsel_sb[:], ssq_sb[:],
                                 start=True, stop=True)
                nc.vector.tensor_scalar_add(seps_sb[:], sq_ps[:], EPS)
                nc.scalar.activation(lnx_sb[:], seps_sb[:], AF.Ln)
                nc.scalar.activation(r_sb[:], lnx_sb[:], AF.Exp, scale=0.5)
                nc.vector.tensor_scalar_add(t1_sb[:], sq_ps[:], 1.0)
                nc.vector.tensor_tensor(den_sb[:], t1_sb[:], r_sb[:],
                                        op=ALU.mult)
                nc.vector.reciprocal(rec_sb[:], den_sb[:])
                nc.vector.tensor_tensor(f_sb[:], sq_ps[:], rec_sb[:],
                                        op=ALU.mult)
                f_ps = psB.tile([128, H * OC], dt.float32, tag="bank",
                                name="f_ps")
                nc.tensor.matmul(f_ps[:], drep_sb[:], f_sb[:],
                                 start=True, stop=True)
                nc.vector.tensor_tensor(v_sb[:], s_sb[:], f_ps[:], op=ALU.mult)
                nc.sync.dma_start(out_d, v_sb[:])

    nc.compile()
    return nc


def _get_program(general_b):
    key = (bool(general_b), NGP_J)
    if key not in _PROG_CACHE:
        _PROG_CACHE[key] = _build_program(key[0])
    return _PROG_CACHE[key]


def _prep_inputs(u_predict, b):
    """Host-side shard + layout transform. Returns (in_maps, general_b)."""
    general_b = bool(np.any(b != 0.0))
    consts = _build_consts()
    u16 = u_predict.astype(np.float16)
    u6 = u16.reshape(NCORES, H, BLOC, IC, OC, D)
    ut = np.ascontiguousarray(u6.transpose(0, 2, 5, 4, 1, 3))
    ut = ut.reshape(NCORES, 128, OC, H, NI)

    extra = {}
    if general_b:
        bm = b.astype(np.float64)
        e = np.exp(bm - bm.max(axis=1, keepdims=True))
        c0 = (e / e.sum(axis=1, keepdims=True)).astype(np.float16)  # [IC, OC]
        c0rep = np.ascontiguousarray(
            np.broadcast_to(c0.T[None, :, :], (128, OC, NI))).astype(
                np.float16)
        bt = b.astype(np.float32).T  # [OC, NI]
        bb0 = np.ascontiguousarray(
            np.repeat(bt[:, None, :], BLOC, axis=1)).reshape(OC * BLOC, NI)
        extra = {"c0rep": c0rep, "bb0": bb0}

    in_maps = []
    for c in range(NCORES):
        m = {"u16": ut[c],
             "sel16": consts["sel16"], "zselrep": consts["zselrep"],
             "base8": consts["base8"],
             "dsel": consts["dsel"], "drep": consts["drep"]}
        m.update(extra)
        in_maps.append(m)
    return in_maps, general_b


def _gather_output(results):
    out = np.empty((B, OC, D), np.float32)
    for c in range(NCORES):
        v = results[c]["vout"]                  # [p=(bl,d), col=(j*2+h)]
        v4 = v.reshape(BLOC, D, OC, H)          # bl, d, j, h
        out[c * BL:(c + 1) * BL] = v4.transpose(3, 0, 2, 1).reshape(
            BL, OC, D)
    return out


def kernel(u_predict, b=None, **kw):
    u_predict = np.asarray(u_predict, dtype=np.float32)
    if b is None:
        b = np.zeros((IC, OC), np.float32)
    b = np.asarray(b, dtype=np.float32)
    in_maps, general_b = _prep_inputs(u_predict, b)
    nc = _get_program(general_b)

    if os.environ.get("BASS_KERNEL_SIM"):
        from concourse.bass_interp import CoreSim
        sim = CoreSim(nc, trace=False)
        for name, arr in in_maps[0].items():
            sim.tensor(name)[:] = arr
        sim.simulate(check_with_hw=False)
        v0 = np.array(sim.tensor("vout"))
        out = np.empty((B, OC, D), np.float32)
        v4 = v0.reshape(BLOC, D, OC, H)
        out[:BL] = v4.transpose(3, 0, 2, 1).reshape(BL, OC, D)
        return out  # NOTE: only core 0 valid in sim mode

    from concourse import bass_utils
    trace = bool(os.environ.get("BASS_KERNEL_TRACE"))
    res = bass_utils.run_bass_kernel_spmd(
        nc, in_maps, core_ids=list(range(NCORES)), trace=trace)
    kernel.last_results = res
    return _gather_output(res.results)



# revision 5
# speedup vs baseline: 1.6816x; 1.6816x over previous
"""AgreementRouting (CapsNet dynamic routing) Trainium2 kernel.

Data-parallel over batch B=128 across 8 cores (B_local=16 per core).

Per core, u lives in SBUF twice, as fp16:
  u16: partition p = b_loc*16 + d   (b_loc in [0,8), d in [0,16))
       free       = (j in [0,10), h in [0,2), i in [0,1152))
  uT:  partition  = i_lo = i % 128
       free       = (h, j, ci = i//128 in [0,9), p = (b_loc, d))
local batch index beta = h*8 + b_loc.

Structure per routing iteration:
  W1: PE accumulating matmuls a_c = sum_j sfat[j].T @ u16[j]    (fp16)
      (sfat = block-diagonal stationary holding the unnormalized s)
  bb += f80 * a_c                   (DVE/GpSimd STT, squash scale f
                                     folded into the logit update)
  e = exp(bb); Z = zselrep.T @ e (PE); lzr = ln(Z); tm = bb - lzr;
  c16 = exp(tm)                     (ACT/DVE, fp16 coupling coeffs)
  cT: PE transpose of c16 chunks -> [i_lo, (ci, j, b)] fp16, one ACT
      copy PSUM->SBUF per h
  W4: weighted sum on PE with uT as the *stationary*:
      s_ps[(b,d), b'] += sum_i uT[i,(b,d)] * cT[i,(j,b')]  (9 chunks)
      then a masked DVE reduce extracts the b'==b diagonal into s.
  squash scale f80 computed entirely in (j,b)-partition layout
      (PE matmul + masked STT), no SBUF-shuffle DMAs.

Numerics vs the fp32 oracle: absmax/scale ~ 5e-4.
"""

import os
import sys

import numpy as np

for _p in ("/opt/trn_rl_repo", "/opt/trn_rl_repo/concourse"):
    if _p not in sys.path and os.path.isdir(_p):
        sys.path.insert(0, _p)

B, IC, OC, D = 128, 1152, 10, 16
NCORES = 8
BL = B // NCORES          # 16 local batch
H = 2                     # halves of local batch
BLOC = BL // H            # 8
NI = IC                   # 1152
NC9 = NI // 128           # 9 i-chunks of 128
EPS = 1e-8
NITER = 3
CHUNKS = [(0, 512), (512, 1024), (1024, 1152)]
TIME_REPS = int(os.environ.get("K_TIME_REPS", "1"))  # whole-program reps

_PROG_CACHE = {}


def _build_consts():
    """Host-side constant selector/mask matrices."""
    # zselrep[(j,b), (j2,b2)] = 1 if b==b2               -> [80, 80] f32
    zselrep = np.zeros((OC * BLOC, OC * BLOC), np.float32)
    for j in range(OC):
        for b in range(BLOC):
            for j2 in range(OC):
                zselrep[j * BLOC + b, j2 * BLOC + b] = 1.0
    # base8[(b,d), b2] = 1 if b==b2                      -> [128, 8] f32
    base8 = np.zeros((BLOC * D, BLOC), np.float32)
    for b in range(BLOC):
        base8[b * D:(b + 1) * D, b] = 1.0
    drep = np.ascontiguousarray(base8.T)
    # b82a[(b,d), (j,b2)] = 1 if b==b2                   -> [128, 80]
    b82a = np.tile(base8, (1, OC)).astype(np.float32)
    # jmask[(j,b), j2] = 1 if j==j2                      -> [80, 10] f32
    jmask = np.zeros((OC * BLOC, OC), np.float32)
    for j in range(OC):
        jmask[j * BLOC:(j + 1) * BLOC, j] = 1.0
    ident80 = np.eye(OC * BLOC, dtype=np.float16)
    return dict(zselrep=zselrep, base8=base8, drep=drep,
                b82a32=b82a, b82a16=b82a.astype(np.float16),
                jmask=jmask, ident80=ident80)


def _build_program(general_b):
    import concourse.bacc as bacc
    import concourse.mybir as mybir
    import concourse.tile as tile

    dt = mybir.dt
    AF = mybir.ActivationFunctionType
    ALU = mybir.AluOpType
    AX = mybir.AxisListType

    # Force a single shared ACT table (Exp+Ln+Copy+Identity in one set) so
    # the table-load pass emits one load instead of thrashing per func.
    from concourse import hw_specs as _hws
    _orig_tabs = _hws.get_activation_tables
    _keep = "natural_log_exp_and_others"

    def _patched_tabs(arch, __orig=_orig_tabs, __keep=_keep):
        tabs = __orig(arch)
        return {n: (s if n == __keep else set()) for n, s in tabs.items()}

    bacc.get_activation_tables = _patched_tabs

    nc = bacc.Bacc("TRN2", target_bir_lowering=False, debug=False)

    # ---- DRAM I/O ----
    u16_d = nc.dram_tensor("u16", [128, OC, H, NI], dt.float16,
                           kind="ExternalInput").ap()
    uT_d = nc.dram_tensor("uT", [128, H, OC, NC9 * 128], dt.float16,
                          kind="ExternalInput").ap()
    zselrep_d = nc.dram_tensor("zselrep", [OC * BLOC, OC * BLOC], dt.float32,
                               kind="ExternalInput").ap()
    base8_d = nc.dram_tensor("base8", [BLOC * D, BLOC], dt.float32,
                             kind="ExternalInput").ap()
    drep_d = nc.dram_tensor("drep", [BLOC, BLOC * D], dt.float32,
                            kind="ExternalInput").ap()
    b82a32_d = nc.dram_tensor("b82a32", [BLOC * D, OC * BLOC], dt.float32,
                              kind="ExternalInput").ap()
    b82a16_d = nc.dram_tensor("b82a16", [BLOC * D, OC * BLOC], dt.float16,
                              kind="ExternalInput").ap()
    jmask_d = nc.dram_tensor("jmask", [OC * BLOC, OC], dt.float32,
                             kind="ExternalInput").ap()
    ident80_d = nc.dram_tensor("ident80", [OC * BLOC, OC * BLOC], dt.float16,
                               kind="ExternalInput").ap()
    if general_b:
        c0_d = nc.dram_tensor("c0rep", [128, OC, NI], dt.float16,
                              kind="ExternalInput").ap()
        bb0_d = nc.dram_tensor("bb0", [OC * BLOC, NI], dt.float32,
                               kind="ExternalInput").ap()
    out_d = nc.dram_tensor("vout", [128, 2 * OC], dt.float32,
                           kind="ExternalOutput").ap()

    # ---- static SBUF ----
    def sb(name, shape, dtype):
        return nc.alloc_sbuf_tensor(name, list(shape), dtype).ap()

    u16 = sb("u16_sb", [128, OC * H * NI], dt.float16)       # 46KB/part
    uT = sb("uT_sb", [128, H * OC * NC9 * 128], dt.float16)  # 46KB/part
    zselrep_sb = sb("zselrep_sb", [OC * BLOC, OC * BLOC], dt.float32)
    base8_sb = sb("base8_sb", [BLOC * D, BLOC], dt.float32)
    drep_sb = sb("drep_sb", [BLOC, BLOC * D], dt.float32)
    b82a32_sb = sb("b82a32_sb", [BLOC * D, OC * BLOC], dt.float32)
    b82a16_sb = sb("b82a16_sb", [BLOC * D, OC * BLOC], dt.float16)
    jmask_sb = sb("jmask_sb", [OC * BLOC, OC], dt.float32)
    ident80_sb = sb("ident80_sb", [OC * BLOC, OC * BLOC], dt.float16)
    bb = [sb(f"bbsb{h}", [OC * BLOC, NI], dt.float32) for h in range(H)]
    sfat = [[sb(f"sfat{j}_{h}", [128, OC * BLOC], dt.float16)
             for h in range(H)] for j in range(OC)]
    f80 = [sb(f"f80_{h}", [OC * BLOC, 1], dt.float32) for h in range(H)]
    s_sb = sb("s_sb", [128, H * OC], dt.float32)
    cT_sb = [sb(f"cT_sb{h}", [128, NC9 * OC * BLOC], dt.float16)
             for h in range(H)]
    # mini-squash scratch (per h), all in [80, *] layout
    ssqh = [sb(f"ssqh{h}", [128, OC], dt.float16) for h in range(H)]
    jscr = [sb(f"jscr{h}", [OC * BLOC, OC], dt.float32) for h in range(H)]
    sq80 = [sb(f"sq80_{h}", [OC * BLOC, 1], dt.float32) for h in range(H)]
    seps80 = [sb(f"seps80_{h}", [OC * BLOC, 1], dt.float32) for h in range(H)]
    lnx80 = [sb(f"lnx80_{h}", [OC * BLOC, 1], dt.float32) for h in range(H)]
    r80 = [sb(f"r80_{h}", [OC * BLOC, 1], dt.float32) for h in range(H)]
    t180 = [sb(f"t180_{h}", [OC * BLOC, 1], dt.float32) for h in range(H)]
    den80 = [sb(f"den80_{h}", [OC * BLOC, 1], dt.float32) for h in range(H)]
    rec80 = [sb(f"rec80_{h}", [OC * BLOC, 1], dt.float32) for h in range(H)]
    # W4 extraction scratch
    mskd = [sb(f"mskd{h}", [128, OC * BLOC], dt.float32) for h in range(H)]
    # final squash scratch
    ssq_sb = sb("ssq_sb", [128, H * OC], dt.float32)
    seps_sb = sb("seps_sb", [BLOC, H * OC], dt.float32)
    lnx_sb = sb("lnx_sb", [BLOC, H * OC], dt.float32)
    r_sb = sb("r_sb", [BLOC, H * OC], dt.float32)
    t1_sb = sb("t1_sb", [BLOC, H * OC], dt.float32)
    den_sb = sb("den_sb", [BLOC, H * OC], dt.float32)
    rec_sb = sb("rec_sb", [BLOC, H * OC], dt.float32)
    f_sb = sb("f_sb", [BLOC, H * OC], dt.float32)
    v_sb = sb("v_sb", [128, H * OC], dt.float32)

    def uview(j, h):
        off = (j * H + h) * NI
        return u16[:, off:off + NI]

    def uTview(h, j, ci):
        off = ((h * OC + j) * NC9 + ci) * 128
        return uT[:, off:off + 128]

    with tile.TileContext(nc) as tc:
        from contextlib import ExitStack
        with ExitStack() as ctx:
            psA = ctx.enter_context(
                tc.tile_pool(name="psA", bufs=2, space="PSUM"))
            psB = ctx.enter_context(
                tc.tile_pool(name="psB", bufs=2, space="PSUM"))
            psT = ctx.enter_context(
                tc.tile_pool(name="psT", bufs=2, space="PSUM"))
            psS = ctx.enter_context(
                tc.tile_pool(name="psS", bufs=2, space="PSUM"))
            sc = ctx.enter_context(
                tc.tile_pool(name="sc", bufs=int(os.environ.get("K_SCBUFS", "3"))))
            ec = ctx.enter_context(
                tc.tile_pool(name="ec", bufs=int(os.environ.get("K_ECBUFS", "4"))))

            for _rep in range(TIME_REPS):
                # ---- loads: consts first (tiny), then u16, then uT ----
                nc.sync.dma_start(zselrep_sb[:], zselrep_d)
                nc.sync.dma_start(base8_sb[:], base8_d)
                nc.sync.dma_start(drep_sb[:], drep_d)
                nc.sync.dma_start(b82a32_sb[:], b82a32_d)
                nc.sync.dma_start(b82a16_sb[:], b82a16_d)
                nc.sync.dma_start(jmask_sb[:], jmask_d)
                nc.sync.dma_start(ident80_sb[:], ident80_d)
                for j in range(OC):
                    nc.sync.dma_start(
                        u16[:, j * H * NI:(j + 1) * H * NI], u16_d[:, j, :, :])
                for h in range(H):
                    for j in range(OC):
                        off = (h * OC + j) * NC9 * 128
                        nc.sync.dma_start(
                            uT[:, off:off + NC9 * 128], uT_d[:, h, j, :])

                # ---- init bb and sfat ----
                for h in range(H):
                    if general_b:
                        nc.sync.dma_start(bb[h][:], bb0_d)
                    else:
                        nc.gpsimd.memset(bb[h][:], 0.0)
                for j in range(OC):
                    for h in range(H):
                        nc.gpsimd.memset(sfat[j][h][:], 0.0)

                def build_sfat(j, h):
                    col = 2 * j + h
                    nc.vector.tensor_scalar(
                        out=sfat[j][h][:, j * BLOC:(j + 1) * BLOC],
                        in0=base8_sb[:],
                        scalar1=s_sb[:, col:col + 1],
                        scalar2=None,
                        op0=ALU.mult)

                def mini_squash(h):
                    """f80[h] <- squash scale, computed in [80,*] layout."""
                    s_h = s_sb[:, h::2]  # [128, OC] strided view
                    nc.vector.tensor_tensor(ssqh[h][:], s_h, s_h, op=ALU.mult)
                    sq_ps = psB.tile([OC * BLOC, OC], dt.float32, tag="bank",
                                     name="sq80_ps")
                    nc.tensor.matmul(sq_ps[:], b82a16_sb[:], ssqh[h][:],
                                     start=True, stop=True)
                    nc.vector.scalar_tensor_tensor(
                        out=jscr[h][:], in0=sq_ps[:], scalar=1.0,
                        in1=jmask_sb[:], op0=ALU.mult, op1=ALU.mult,
                        accum_out=sq80[h][:])
                    nc.vector.tensor_scalar_add(seps80[h][:], sq80[h][:], EPS)
                    nc.scalar.activation(lnx80[h][:], seps80[h][:], AF.Ln)
                    nc.scalar.activation(r80[h][:], lnx80[h][:], AF.Exp,
                                         scale=0.5)
                    nc.vector.tensor_scalar_add(t180[h][:], sq80[h][:], 1.0)
                    nc.vector.tensor_tensor(den80[h][:], t180[h][:], r80[h][:],
                                            op=ALU.mult)
                    nc.vector.reciprocal(rec80[h][:], den80[h][:])
                    nc.vector.tensor_tensor(f80[h][:], sq80[h][:],
                                            rec80[h][:], op=ALU.mult)

                # ---- init s0 ----
                if general_b:
                    c0_sb = sc.tile([128, OC * NI], dt.float16, tag="c0",
                                    name="c0_sb", bufs=1)
                    nc.sync.dma_start(c0_sb[:], c0_d)
                    for j in range(OC):
                        for h in range(H):
                            col = 2 * j + h
                            scr = sc.tile([128, NI], dt.float16, tag="scr",
                                          name="scr")
                            nc.vector.scalar_tensor_tensor(
                                out=scr[:], in0=uview(j, h), scalar=1.0,
                                in1=c0_sb[:, j * NI:(j + 1) * NI],
                                op0=ALU.mult, op1=ALU.mult,
                                accum_out=s_sb[:, col:col + 1])
                else:
                    # split the 20 (j,h) reduction units across DVE/ACT/GpSimd
                    units = [(j, h) for j in range(OC) for h in range(H)]
                    for idx, (j, h) in enumerate(units):
                        col = 2 * j + h
                        eng = ("act", "dve")[idx % 2]
                        if eng == "dve":
                            nc.vector.reduce_sum(
                                s_sb[:, col:col + 1], uview(j, h), axis=AX.X)
                        else:
                            scr = sc.tile([128, NI], dt.float16, tag="scr",
                                          name="scr")
                            nc.scalar.activation(
                                scr[:], uview(j, h), AF.Identity,
                                accum_out=s_sb[:, col:col + 1])
                if not general_b:
                    nc.vector.tensor_scalar_mul(s_sb[:], s_sb[:], 1.0 / OC)
                for j in range(OC):
                    for h in range(H):
                        build_sfat(j, h)
                for h in range(H):
                    mini_squash(h)

                # ---- routing iterations ----
                for it in range(NITER):
                    last = it == NITER - 1
                    # stage-major emission: each engine streams all 6 chunks
                    # (2h x 3c) back-to-back, so cross-engine latency amortizes.
                    hc = [(h, ci) for h in range(H) for ci in range(len(CHUNKS))]
                    a_t = {}
                    for (h, ci) in hc:
                        c0, c1 = CHUNKS[ci]
                        w = c1 - c0
                        a_c = psB.tile([OC * BLOC, 512], dt.float32,
                                       tag="bank", name="a_c")
                        for j in range(OC):
                            nc.tensor.matmul(
                                a_c[:, :w], sfat[j][h],
                                uview(j, h)[:, c0:c1],
                                start=(j == 0), stop=(j == OC - 1))
                        a_t[(h, ci)] = a_c
                    for (h, ci) in hc:
                        c0, c1 = CHUNKS[ci]
                        eng = nc.gpsimd if ci == 2 else nc.vector
                        eng.scalar_tensor_tensor(
                            out=bb[h][:, c0:c1], in0=a_t[(h, ci)][:, :c1 - c0],
                            scalar=f80[h][:, 0:1], in1=bb[h][:, c0:c1],
                            op0=ALU.mult, op1=ALU.add)
                    e_t = {}
                    for (h, ci) in hc:
                        c0, c1 = CHUNKS[ci]
                        e_c = ec.tile([OC * BLOC, 512], dt.float32,
                                      tag="e", name="e_c")
                        nc.scalar.activation(e_c[:, :c1 - c0], bb[h][:, c0:c1],
                                             AF.Exp)
                        e_t[(h, ci)] = e_c
                    z_t = {}
                    for (h, ci) in hc:
                        c0, c1 = CHUNKS[ci]
                        z_c = psA.tile([OC * BLOC, 512], dt.float32,
                                       tag="big", name="z_c")
                        nc.tensor.matmul(z_c[:, :c1 - c0], zselrep_sb[:],
                                         e_t[(h, ci)][:, :c1 - c0],
                                         start=True, stop=True)
                        z_t[(h, ci)] = z_c
                    lzr_t = {}
                    for (h, ci) in hc:
                        c0, c1 = CHUNKS[ci]
                        lzr_c = ec.tile([OC * BLOC, 512], dt.float32,
                                        tag="lzr", name="lzr_c")
                        nc.scalar.activation(lzr_c[:, :c1 - c0],
                                             z_t[(h, ci)][:, :c1 - c0], AF.Ln)
                        lzr_t[(h, ci)] = lzr_c
                    tm_t = {}
                    for (h, ci) in hc:
                        c0, c1 = CHUNKS[ci]
                        tm_c = ec.tile([OC * BLOC, 512], dt.float32,
                                       tag="tm", name="tm_c")
                        eng = nc.gpsimd if ci == 2 else nc.vector
                        eng.tensor_tensor(tm_c[:, :c1 - c0],
                                          bb[h][:, c0:c1],
                                          lzr_t[(h, ci)][:, :c1 - c0],
                                          op=ALU.subtract)
                        tm_t[(h, ci)] = tm_c
                    c16 = {}
                    for h in range(H):
                        c16[h] = ec.tile([OC * BLOC, NI], dt.float16,
                                         tag="c16", name="c16")
                    for (h, ci) in hc:
                        c0, c1 = CHUNKS[ci]
                        nc.scalar.activation(c16[h][:, c0:c1],
                                             tm_t[(h, ci)][:, :c1 - c0], AF.Exp)

                    # ---- W4: transpose c16, then PE weighted-sum with uT
                    # stationary; masked reduce extracts the diagonal ----
                    ct_ps = {}
                    for h in range(H):
                        ct_ps[h] = psT.tile([128, NC9 * OC * BLOC], dt.float16,
                                            tag="ct", name="ct_ps")
                        for ci in range(NC9):
                            nc.tensor.transpose(
                                ct_ps[h][:, ci * 80:(ci + 1) * 80],
                                c16[h][:, ci * 128:(ci + 1) * 128],
                                ident80_sb[:])
                        nc.scalar.copy(cT_sb[h][:], ct_ps[h][:])
                    for h in range(H):
                        s_ps = psS.tile([128, OC * BLOC], dt.float32,
                                        tag="sps", name="s_ps")
                        for j in range(OC):
                            for ci in range(NC9):
                                nc.tensor.matmul(
                                    s_ps[:, j * BLOC:(j + 1) * BLOC],
                                    uTview(h, j, ci),
                                    cT_sb[h][:, ci * 80 + j * BLOC:
                                             ci * 80 + (j + 1) * BLOC],
                                    start=(ci == 0), stop=(ci == NC9 - 1))
                        nc.vector.tensor_tensor(mskd[h][:], s_ps[:],
                                                b82a32_sb[:], op=ALU.mult)
                        nc.vector.reduce_sum(
                            s_sb[:, h::2],
                            mskd[h][:].rearrange("p (j b) -> p j b", j=OC),
                            axis=AX.X)
                        if not last:
                            for j in range(OC):
                                build_sfat(j, h)
                            mini_squash(h)

                # ---- final squash -> v ----
                nc.vector.tensor_tensor(ssq_sb[:], s_sb[:], s_sb[:], op=ALU.mult)
                sq_ps = psB.tile([BLOC, H * OC], dt.float32, tag="bank",
                                 name="sq_ps")
                nc.tensor.matmul(sq_ps[:], base8_sb[:], ssq_sb[:],
                                 start=True, stop=True)
                nc.vector.tensor_scalar_add(seps_sb[:], sq_ps[:], EPS)
                nc.scalar.activation(lnx_sb[:], seps_sb[:], AF.Ln)
                nc.scalar.activation(r_sb[:], lnx_sb[:], AF.Exp, scale=0.5)
                nc.vector.tensor_scalar_add(t1_sb[:], sq_ps[:], 1.0)
                nc.vector.tensor_tensor(den_sb[:], t1_sb[:], r_sb[:],
                                        op=ALU.mult)
                nc.vector.reciprocal(rec_sb[:], den_sb[:])
                nc.vector.tensor_tensor(f_sb[:], sq_ps[:], rec_sb[:],
                                        op=ALU.mult)
                f_ps = psB.tile([128, H * OC], dt.float32, tag="bank",
                                name="f_ps")
                nc.tensor.matmul(f_ps[:], drep_sb[:], f_sb[:],
                                 start=True, stop=True)
                nc.vector.tensor_tensor(v_sb[:], s_sb[:], f_ps[:], op=ALU.mult)
                nc.sync.dma_start(out_d, v_sb[:])

    nc.compile()
    return nc


def _get_program(general_b):
    key = bool(general_b)
    if key not in _PROG_CACHE:
        _PROG_CACHE[key] = _build_program(key)
    return _PROG_CACHE[key]


def _prep_inputs(u_predict, b):
    """Host-side shard + layout transform. Returns (in_maps, general_b)."""
    general_b = bool(np.any(b != 0.0))
    consts = _build_consts()
    u16 = u_predict.astype(np.float16)
    u6 = u16.reshape(NCORES, H, BLOC, IC, OC, D)
    ut = np.ascontiguousarray(u6.transpose(0, 2, 5, 4, 1, 3))
    ut = ut.reshape(NCORES, 128, OC, H, NI)
    # uT[c, i_lo, h, j, ci*128 + p] = ut[c, p, j, h, ci*128 + i_lo]
    u5 = ut.reshape(NCORES, 128, OC, H, NC9, 128)
    uTt = np.ascontiguousarray(u5.transpose(0, 5, 3, 2, 4, 1))
    uTt = uTt.reshape(NCORES, 128, H, OC, NC9 * 128)

    extra = {}
    if general_b:
        bm = b.astype(np.float64)
        e = np.exp(bm - bm.max(axis=1, keepdims=True))
        c0 = (e / e.sum(axis=1, keepdims=True)).astype(np.float16)  # [IC, OC]
        c0rep = np.ascontiguousarray(
            np.broadcast_to(c0.T[None, :, :], (128, OC, NI))).astype(
                np.float16)
        bt = b.astype(np.float32).T  # [OC, NI]
        bb0 = np.ascontiguousarray(
            np.repeat(bt[:, None, :], BLOC, axis=1)).reshape(OC * BLOC, NI)
        extra = {"c0rep": c0rep, "bb0": bb0}

    in_maps = []
    for c in range(NCORES):
        m = {"u16": ut[c], "uT": uTt[c]}
        m.update(consts)
        m.update(extra)
        in_maps.append(m)
    return in_maps, general_b


def _gather_output(results):
    out = np.empty((B, OC, D), np.float32)
    for c in range(NCORES):
        v = results[c]["vout"]                  # [p=(bl,d), col=(j*2+h)]
        v4 = v.reshape(BLOC, D, OC, H)          # bl, d, j, h
        out[c * BL:(c + 1) * BL] = v4.transpose(3, 0, 2, 1).reshape(
            BL, OC, D)
    return out


def kernel(u_predict, b=None, **kw):
    u_predict = np.asarray(u_predict, dtype=np.float32)
    if b is None:
        b = np.zeros((IC, OC), np.float32)
    b = np.asarray(b, dtype=np.float32)
    in_maps, general_b = _prep_inputs(u_predict, b)
    nc = _get_program(general_b)

    if os.environ.get("BASS_KERNEL_SIM"):
        from concourse.bass_interp import CoreSim
        sim = CoreSim(nc, trace=False)
        for name, arr in in_maps[0].items():
            sim.tensor(name)[:] = arr
        sim.simulate(check_with_hw=False)
        v0 = np.array(sim.tensor("vout"))
        out = np.empty((B, OC, D), np.float32)
        v4 = v0.reshape(BLOC, D, OC, H)
        out[:BL] = v4.transpose(3, 0, 2, 1).reshape(BL, OC, D)
        return out  # NOTE: only core 0 valid in sim mode

    from concourse import bass_utils
    trace = bool(os.environ.get("BASS_KERNEL_TRACE"))
    res = bass_utils.run_bass_kernel_spmd(
        nc, in_maps, core_ids=list(range(NCORES)), trace=trace)
    kernel.last_results = res
    return _gather_output(res.results)


# revision 6
# speedup vs baseline: 1.6892x; 1.0045x over previous
"""AgreementRouting (CapsNet dynamic routing) Trainium2 kernel.

Data-parallel over batch B=128 across 8 cores (B_local=16 per core).

Per core, u lives in SBUF twice, as fp16:
  u16: partition p = b_loc*16 + d   (b_loc in [0,8), d in [0,16))
       free       = (j in [0,10), h in [0,2), i in [0,1152))
  uT:  partition  = i_lo = i % 128
       free       = (h, j, ci = i//128 in [0,9), p = (b_loc, d))
local batch index beta = h*8 + b_loc.

Structure per routing iteration:
  W1: PE accumulating matmuls a_c = sum_j sfat[j].T @ u16[j]    (fp16)
      (sfat = block-diagonal stationary holding the unnormalized s)
  bb += f80 * a_c                   (DVE/GpSimd STT, squash scale f
                                     folded into the logit update)
  e = exp(bb); Z = zselrep.T @ e (PE); lzr = ln(Z); tm = bb - lzr;
  c16 = exp(tm)                     (ACT/DVE, fp16 coupling coeffs)
  cT: PE transpose of c16 chunks -> [i_lo, (ci, j, b)] fp16, one ACT
      copy PSUM->SBUF per h
  W4: weighted sum on PE with uT as the *stationary*:
      s_ps[(b,d), b'] += sum_i uT[i,(b,d)] * cT[i,(j,b')]  (9 chunks)
      then a masked DVE reduce extracts the b'==b diagonal into s.
  squash scale f80 computed entirely in (j,b)-partition layout
      (PE matmul + masked STT), no SBUF-shuffle DMAs.

Numerics vs the fp32 oracle: absmax/scale ~ 5e-4.
"""

import os
import sys

import numpy as np

for _p in ("/opt/trn_rl_repo", "/opt/trn_rl_repo/concourse"):
    if _p not in sys.path and os.path.isdir(_p):
        sys.path.insert(0, _p)

B, IC, OC, D = 128, 1152, 10, 16
NCORES = 8
BL = B // NCORES          # 16 local batch
H = 2                     # halves of local batch
BLOC = BL // H            # 8
NI = IC                   # 1152
NC9 = NI // 128           # 9 i-chunks of 128
EPS = 1e-8
NITER = 3
CHUNKS = [(0, 512), (512, 1024), (1024, 1152)]
TIME_REPS = int(os.environ.get("K_TIME_REPS", "1"))  # whole-program reps

_PROG_CACHE = {}


def _build_consts():
    """Host-side constant selector/mask matrices."""
    # zselrep[(j,b), (j2,b2)] = 1 if b==b2               -> [80, 80] f32
    zselrep = np.zeros((OC * BLOC, OC * BLOC), np.float32)
    for j in range(OC):
        for b in range(BLOC):
            for j2 in range(OC):
                zselrep[j * BLOC + b, j2 * BLOC + b] = 1.0
    # base8[(b,d), b2] = 1 if b==b2                      -> [128, 8] f32
    base8 = np.zeros((BLOC * D, BLOC), np.float32)
    for b in range(BLOC):
        base8[b * D:(b + 1) * D, b] = 1.0
    drep = np.ascontiguousarray(base8.T)
    # b82a[(b,d), (j,b2)] = 1 if b==b2                   -> [128, 80]
    b82a = np.tile(base8, (1, OC)).astype(np.float32)
    # jmask[(j,b), j2] = 1 if j==j2                      -> [80, 10] f32
    jmask = np.zeros((OC * BLOC, OC), np.float32)
    for j in range(OC):
        jmask[j * BLOC:(j + 1) * BLOC, j] = 1.0
    ident80 = np.eye(OC * BLOC, dtype=np.float16)
    return dict(zselrep=zselrep, base8=base8, drep=drep,
                b82a32=b82a, b82a16=b82a.astype(np.float16),
                jmask=jmask, ident80=ident80)


def _build_program(general_b):
    import concourse.bacc as bacc
    import concourse.mybir as mybir
    import concourse.tile as tile

    dt = mybir.dt
    AF = mybir.ActivationFunctionType
    ALU = mybir.AluOpType
    AX = mybir.AxisListType

    # Force a single shared ACT table (Exp+Ln+Copy+Identity in one set) so
    # the table-load pass emits one load instead of thrashing per func.
    from concourse import hw_specs as _hws
    _orig_tabs = _hws.get_activation_tables
    _keep = "natural_log_exp_and_others"

    def _patched_tabs(arch, __orig=_orig_tabs, __keep=_keep):
        tabs = __orig(arch)
        return {n: (s if n == __keep else set()) for n, s in tabs.items()}

    bacc.get_activation_tables = _patched_tabs

    nc = bacc.Bacc("TRN2", target_bir_lowering=False, debug=False)

    # ---- DRAM I/O ----
    u16_d = nc.dram_tensor("u16", [128, OC, H, NI], dt.float16,
                           kind="ExternalInput").ap()
    uT_d = nc.dram_tensor("uT", [128, H, OC, NC9 * 128], dt.float16,
                          kind="ExternalInput").ap()
    zselrep_d = nc.dram_tensor("zselrep", [OC * BLOC, OC * BLOC], dt.float32,
                               kind="ExternalInput").ap()
    base8_d = nc.dram_tensor("base8", [BLOC * D, BLOC], dt.float32,
                             kind="ExternalInput").ap()
    drep_d = nc.dram_tensor("drep", [BLOC, BLOC * D], dt.float32,
                            kind="ExternalInput").ap()
    b82a32_d = nc.dram_tensor("b82a32", [BLOC * D, OC * BLOC], dt.float32,
                              kind="ExternalInput").ap()
    b82a16_d = nc.dram_tensor("b82a16", [BLOC * D, OC * BLOC], dt.float16,
                              kind="ExternalInput").ap()
    jmask_d = nc.dram_tensor("jmask", [OC * BLOC, OC], dt.float32,
                             kind="ExternalInput").ap()
    ident80_d = nc.dram_tensor("ident80", [OC * BLOC, OC * BLOC], dt.float16,
                               kind="ExternalInput").ap()
    if general_b:
        c0_d = nc.dram_tensor("c0rep", [128, OC, NI], dt.float16,
                              kind="ExternalInput").ap()
        bb0_d = nc.dram_tensor("bb0", [OC * BLOC, NI], dt.float32,
                               kind="ExternalInput").ap()
    out_d = nc.dram_tensor("vout", [128, 2 * OC], dt.float32,
                           kind="ExternalOutput").ap()

    # ---- static SBUF ----
    def sb(name, shape, dtype):
        return nc.alloc_sbuf_tensor(name, list(shape), dtype).ap()

    u16 = sb("u16_sb", [128, OC * H * NI], dt.float16)       # 46KB/part
    uT = sb("uT_sb", [128, H * OC * NC9 * 128], dt.float16)  # 46KB/part
    zselrep_sb = sb("zselrep_sb", [OC * BLOC, OC * BLOC], dt.float32)
    base8_sb = sb("base8_sb", [BLOC * D, BLOC], dt.float32)
    drep_sb = sb("drep_sb", [BLOC, BLOC * D], dt.float32)
    b82a32_sb = sb("b82a32_sb", [BLOC * D, OC * BLOC], dt.float32)
    b82a16_sb = sb("b82a16_sb", [BLOC * D, OC * BLOC], dt.float16)
    jmask_sb = sb("jmask_sb", [OC * BLOC, OC], dt.float32)
    ident80_sb = sb("ident80_sb", [OC * BLOC, OC * BLOC], dt.float16)
    bb = [sb(f"bbsb{h}", [OC * BLOC, NI], dt.float32) for h in range(H)]
    sfat = [[sb(f"sfat{j}_{h}", [128, OC * BLOC], dt.float16)
             for h in range(H)] for j in range(OC)]
    f80 = [sb(f"f80_{h}", [OC * BLOC, 1], dt.float32) for h in range(H)]
    s_sb = sb("s_sb", [128, H * OC], dt.float32)
    cT_sb = [sb(f"cT_sb{h}", [128, NC9 * OC * BLOC], dt.float16)
             for h in range(H)]
    # mini-squash scratch (per h), all in [80, *] layout
    ssqh = [sb(f"ssqh{h}", [128, OC], dt.float16) for h in range(H)]
    jscr = [sb(f"jscr{h}", [OC * BLOC, OC], dt.float32) for h in range(H)]
    sq80 = [sb(f"sq80_{h}", [OC * BLOC, 1], dt.float32) for h in range(H)]
    seps80 = [sb(f"seps80_{h}", [OC * BLOC, 1], dt.float32) for h in range(H)]
    lnx80 = [sb(f"lnx80_{h}", [OC * BLOC, 1], dt.float32) for h in range(H)]
    r80 = [sb(f"r80_{h}", [OC * BLOC, 1], dt.float32) for h in range(H)]
    t180 = [sb(f"t180_{h}", [OC * BLOC, 1], dt.float32) for h in range(H)]
    den80 = [sb(f"den80_{h}", [OC * BLOC, 1], dt.float32) for h in range(H)]
    rec80 = [sb(f"rec80_{h}", [OC * BLOC, 1], dt.float32) for h in range(H)]
    # W4 extraction scratch
    mskd = [sb(f"mskd{h}", [128, OC * BLOC], dt.float32) for h in range(H)]
    # final squash scratch
    ssq_sb = sb("ssq_sb", [128, H * OC], dt.float32)
    seps_sb = sb("seps_sb", [BLOC, H * OC], dt.float32)
    lnx_sb = sb("lnx_sb", [BLOC, H * OC], dt.float32)
    r_sb = sb("r_sb", [BLOC, H * OC], dt.float32)
    t1_sb = sb("t1_sb", [BLOC, H * OC], dt.float32)
    den_sb = sb("den_sb", [BLOC, H * OC], dt.float32)
    rec_sb = sb("rec_sb", [BLOC, H * OC], dt.float32)
    f_sb = sb("f_sb", [BLOC, H * OC], dt.float32)
    v_sb = sb("v_sb", [128, H * OC], dt.float32)

    def uview(j, h):
        off = (j * H + h) * NI
        return u16[:, off:off + NI]

    def uTview(h, j, ci):
        off = ((h * OC + j) * NC9 + ci) * 128
        return uT[:, off:off + 128]

    with tile.TileContext(nc) as tc:
        from contextlib import ExitStack
        with ExitStack() as ctx:
            psA = ctx.enter_context(
                tc.tile_pool(name="psA", bufs=2, space="PSUM"))
            psB = ctx.enter_context(
                tc.tile_pool(name="psB", bufs=2, space="PSUM"))
            psT = ctx.enter_context(
                tc.tile_pool(name="psT", bufs=2, space="PSUM"))
            psS = ctx.enter_context(
                tc.tile_pool(name="psS", bufs=2, space="PSUM"))
            sc = ctx.enter_context(
                tc.tile_pool(name="sc", bufs=int(os.environ.get("K_SCBUFS", "3"))))
            ec = ctx.enter_context(
                tc.tile_pool(name="ec", bufs=int(os.environ.get("K_ECBUFS", "4"))))

            for _rep in range(TIME_REPS):
                # ---- loads: consts first (tiny), then u16, then uT ----
                nc.sync.dma_start(zselrep_sb[:], zselrep_d)
                nc.sync.dma_start(base8_sb[:], base8_d)
                nc.sync.dma_start(drep_sb[:], drep_d)
                nc.sync.dma_start(b82a32_sb[:], b82a32_d)
                nc.sync.dma_start(b82a16_sb[:], b82a16_d)
                nc.sync.dma_start(jmask_sb[:], jmask_d)
                nc.sync.dma_start(ident80_sb[:], ident80_d)
                for j in range(OC):
                    nc.sync.dma_start(
                        u16[:, j * H * NI:(j + 1) * H * NI], u16_d[:, j, :, :])
                for h in range(H):
                    for j in range(OC):
                        off = (h * OC + j) * NC9 * 128
                        nc.sync.dma_start(
                            uT[:, off:off + NC9 * 128], uT_d[:, h, j, :])

                # ---- init bb and sfat ----
                for h in range(H):
                    if general_b:
                        nc.sync.dma_start(bb[h][:], bb0_d)
                    else:
                        nc.gpsimd.memset(bb[h][:], 0.0)
                for j in range(OC):
                    for h in range(H):
                        nc.gpsimd.memset(sfat[j][h][:], 0.0)

                def build_sfat(j, h):
                    col = 2 * j + h
                    nc.vector.tensor_scalar(
                        out=sfat[j][h][:, j * BLOC:(j + 1) * BLOC],
                        in0=base8_sb[:],
                        scalar1=s_sb[:, col:col + 1],
                        scalar2=None,
                        op0=ALU.mult)

                def mini_squash(h):
                    """f80[h] <- squash scale, computed in [80,*] layout."""
                    s_h = s_sb[:, h::2]  # [128, OC] strided view
                    nc.vector.tensor_tensor(ssqh[h][:], s_h, s_h, op=ALU.mult)
                    sq_ps = psB.tile([OC * BLOC, OC], dt.float32, tag="bank",
                                     name="sq80_ps")
                    nc.tensor.matmul(sq_ps[:], b82a16_sb[:], ssqh[h][:],
                                     start=True, stop=True)
                    nc.vector.scalar_tensor_tensor(
                        out=jscr[h][:], in0=sq_ps[:], scalar=1.0,
                        in1=jmask_sb[:], op0=ALU.mult, op1=ALU.mult,
                        accum_out=sq80[h][:])
                    nc.vector.tensor_scalar_add(seps80[h][:], sq80[h][:], EPS)
                    nc.scalar.activation(lnx80[h][:], seps80[h][:], AF.Ln)
                    nc.scalar.activation(r80[h][:], lnx80[h][:], AF.Exp,
                                         scale=0.5)
                    nc.vector.tensor_scalar_add(t180[h][:], sq80[h][:], 1.0)
                    nc.vector.tensor_tensor(den80[h][:], t180[h][:], r80[h][:],
                                            op=ALU.mult)
                    nc.vector.reciprocal(rec80[h][:], den80[h][:])
                    nc.vector.tensor_tensor(f80[h][:], sq80[h][:],
                                            rec80[h][:], op=ALU.mult)

                # ---- init s0 ----
                if general_b:
                    c0_sb = sc.tile([128, OC * NI], dt.float16, tag="c0",
                                    name="c0_sb", bufs=1)
                    nc.sync.dma_start(c0_sb[:], c0_d)
                    for j in range(OC):
                        for h in range(H):
                            col = 2 * j + h
                            scr = sc.tile([128, NI], dt.float16, tag="scr",
                                          name="scr")
                            nc.vector.scalar_tensor_tensor(
                                out=scr[:], in0=uview(j, h), scalar=1.0,
                                in1=c0_sb[:, j * NI:(j + 1) * NI],
                                op0=ALU.mult, op1=ALU.mult,
                                accum_out=s_sb[:, col:col + 1])
                else:
                    # split the 20 (j,h) reduction units across DVE/ACT/GpSimd
                    units = [(j, h) for j in range(OC) for h in range(H)]
                    for idx, (j, h) in enumerate(units):
                        col = 2 * j + h
                        eng = ("act", "dve")[idx % 2]
                        if eng == "dve":
                            nc.vector.reduce_sum(
                                s_sb[:, col:col + 1], uview(j, h), axis=AX.X)
                        else:
                            scr = sc.tile([128, NI], dt.float16, tag="scr",
                                          name="scr")
                            nc.scalar.activation(
                                scr[:], uview(j, h), AF.Identity,
                                accum_out=s_sb[:, col:col + 1])
                if not general_b:
                    nc.vector.tensor_scalar_mul(s_sb[:], s_sb[:], 1.0 / OC)
                for j in range(OC):
                    for h in range(H):
                        build_sfat(j, h)
                for h in range(H):
                    mini_squash(h)

                # ---- routing iterations ----
                for it in range(NITER):
                    last = it == NITER - 1
                    # stage-major emission: each engine streams all 6 chunks
                    # (2h x 3c) back-to-back, so cross-engine latency amortizes.
                    hc = [(h, ci) for h in range(H) for ci in range(len(CHUNKS))]
                    a_t = {}
                    for (h, ci) in hc:
                        c0, c1 = CHUNKS[ci]
                        w = c1 - c0
                        a_c = psB.tile([OC * BLOC, 512], dt.float32,
                                       tag="bank", name="a_c")
                        for j in range(OC):
                            nc.tensor.matmul(
                                a_c[:, :w], sfat[j][h],
                                uview(j, h)[:, c0:c1],
                                start=(j == 0), stop=(j == OC - 1))
                        a_t[(h, ci)] = a_c
                    for (h, ci) in hc:
                        c0, c1 = CHUNKS[ci]
                        # GPSIMD cannot access PSUM (a_c), so DVE only here
                        nc.vector.scalar_tensor_tensor(
                            out=bb[h][:, c0:c1], in0=a_t[(h, ci)][:, :c1 - c0],
                            scalar=f80[h][:, 0:1], in1=bb[h][:, c0:c1],
                            op0=ALU.mult, op1=ALU.add)
                    e_t = {}
                    for (h, ci) in hc:
                        c0, c1 = CHUNKS[ci]
                        e_c = ec.tile([OC * BLOC, 512], dt.float32,
                                      tag="e", name="e_c")
                        nc.scalar.activation(e_c[:, :c1 - c0], bb[h][:, c0:c1],
                                             AF.Exp)
                        e_t[(h, ci)] = e_c
                    z_t = {}
                    for (h, ci) in hc:
                        c0, c1 = CHUNKS[ci]
                        z_c = psA.tile([OC * BLOC, 512], dt.float32,
                                       tag="big", name="z_c")
                        nc.tensor.matmul(z_c[:, :c1 - c0], zselrep_sb[:],
                                         e_t[(h, ci)][:, :c1 - c0],
                                         start=True, stop=True)
                        z_t[(h, ci)] = z_c
                    lzr_t = {}
                    for (h, ci) in hc:
                        c0, c1 = CHUNKS[ci]
                        lzr_c = ec.tile([OC * BLOC, 512], dt.float32,
                                        tag="lzr", name="lzr_c")
                        nc.scalar.activation(lzr_c[:, :c1 - c0],
                                             z_t[(h, ci)][:, :c1 - c0], AF.Ln)
                        lzr_t[(h, ci)] = lzr_c
                    tm_t = {}
                    for (h, ci) in hc:
                        c0, c1 = CHUNKS[ci]
                        tm_c = ec.tile([OC * BLOC, 512], dt.float32,
                                       tag="tm", name="tm_c")
                        eng = nc.gpsimd if ci == 2 else nc.vector
                        eng.tensor_tensor(tm_c[:, :c1 - c0],
                                          bb[h][:, c0:c1],
                                          lzr_t[(h, ci)][:, :c1 - c0],
                                          op=ALU.subtract)
                        tm_t[(h, ci)] = tm_c
                    c16 = {}
                    for h in range(H):
                        c16[h] = ec.tile([OC * BLOC, NI], dt.float16,
                                         tag="c16", name="c16")
                    for (h, ci) in hc:
                        c0, c1 = CHUNKS[ci]
                        nc.scalar.activation(c16[h][:, c0:c1],
                                             tm_t[(h, ci)][:, :c1 - c0], AF.Exp)

                    # ---- W4: transpose c16, then PE weighted-sum with uT
                    # stationary; masked reduce extracts the diagonal ----
                    ct_ps = {}
                    for h in range(H):
                        ct_ps[h] = psT.tile([128, NC9 * OC * BLOC], dt.float16,
                                            tag="ct", name="ct_ps")
                        for ci in range(NC9):
                            nc.tensor.transpose(
                                ct_ps[h][:, ci * 80:(ci + 1) * 80],
                                c16[h][:, ci * 128:(ci + 1) * 128],
                                ident80_sb[:])
                        nc.scalar.copy(cT_sb[h][:], ct_ps[h][:])
                    for h in range(H):
                        s_ps = psS.tile([128, OC * BLOC], dt.float32,
                                        tag="sps", name="s_ps")
                        for j in range(OC):
                            for ci in range(NC9):
                                nc.tensor.matmul(
                                    s_ps[:, j * BLOC:(j + 1) * BLOC],
                                    uTview(h, j, ci),
                                    cT_sb[h][:, ci * 80 + j * BLOC:
                                             ci * 80 + (j + 1) * BLOC],
                                    start=(ci == 0), stop=(ci == NC9 - 1))
                        nc.vector.tensor_tensor(mskd[h][:], s_ps[:],
                                                b82a32_sb[:], op=ALU.mult)
                        nc.vector.reduce_sum(
                            s_sb[:, h::2],
                            mskd[h][:].rearrange("p (j b) -> p j b", j=OC),
                            axis=AX.X)
                        if not last:
                            for j in range(OC):
                                build_sfat(j, h)
                            mini_squash(h)

                # ---- final squash -> v ----
                nc.vector.tensor_tensor(ssq_sb[:], s_sb[:], s_sb[:], op=ALU.mult)
                sq_ps = psB.tile([BLOC, H * OC], dt.float32, tag="bank",
                                 name="sq_ps")
                nc.tensor.matmul(sq_ps[:], base8_sb[:], ssq_sb[:],
                                 start=True, stop=True)
                nc.vector.tensor_scalar_add(seps_sb[:], sq_ps[:], EPS)
                nc.scalar.activation(lnx_sb[:], seps_sb[:], AF.Ln)
                nc.scalar.activation(r_sb[:], lnx_sb[:], AF.Exp, scale=0.5)
                nc.vector.tensor_scalar_add(t1_sb[:], sq_ps[:], 1.0)
                nc.vector.tensor_tensor(den_sb[:], t1_sb[:], r_sb[:],
                                        op=ALU.mult)
                nc.vector.reciprocal(rec_sb[:], den_sb[:])
                nc.vector.tensor_tensor(f_sb[:], sq_ps[:], rec_sb[:],
                                        op=ALU.mult)
                f_ps = psB.tile([128, H * OC], dt.float32, tag="bank",
                                name="f_ps")
                nc.tensor.matmul(f_ps[:], drep_sb[:], f_sb[:],
                                 start=True, stop=True)
                nc.vector.tensor_tensor(v_sb[:], s_sb[:], f_ps[:], op=ALU.mult)
                nc.sync.dma_start(out_d, v_sb[:])

    nc.compile()
    return nc


def _get_program(general_b):
    key = bool(general_b)
    if key not in _PROG_CACHE:
        _PROG_CACHE[key] = _build_program(key)
    return _PROG_CACHE[key]


def _prep_inputs(u_predict, b):
    """Host-side shard + layout transform. Returns (in_maps, general_b)."""
    general_b = bool(np.any(b != 0.0))
    consts = _build_consts()
    u16 = u_predict.astype(np.float16)
    u6 = u16.reshape(NCORES, H, BLOC, IC, OC, D)
    ut = np.ascontiguousarray(u6.transpose(0, 2, 5, 4, 1, 3))
    ut = ut.reshape(NCORES, 128, OC, H, NI)
    # uT[c, i_lo, h, j, ci*128 + p] = ut[c, p, j, h, ci*128 + i_lo]
    u5 = ut.reshape(NCORES, 128, OC, H, NC9, 128)
    uTt = np.ascontiguousarray(u5.transpose(0, 5, 3, 2, 4, 1))
    uTt = uTt.reshape(NCORES, 128, H, OC, NC9 * 128)

    extra = {}
    if general_b:
        bm = b.astype(np.float64)
        e = np.exp(bm - bm.max(axis=1, keepdims=True))
        c0 = (e / e.sum(axis=1, keepdims=True)).astype(np.float16)  # [IC, OC]
        c0rep = np.ascontiguousarray(
            np.broadcast_to(c0.T[None, :, :], (128, OC, NI))).astype(
                np.float16)
        bt = b.astype(np.float32).T  # [OC, NI]
        bb0 = np.ascontiguousarray(
            np.repeat(bt[:, None, :], BLOC, axis=1)).reshape(OC * BLOC, NI)
        extra = {"c0rep": c0rep, "bb0": bb0}

    in_maps = []
    for c in range(NCORES):
        m = {"u16": ut[c], "uT": uTt[c]}
        m.update(consts)
        m.update(extra)
        in_maps.append(m)
    return in_maps, general_b


def _gather_output(results):
    out = np.empty((B, OC, D), np.float32)
    for c in range(NCORES):
        v = results[c]["vout"]                  # [p=(bl,d), col=(j*2+h)]
        v4 = v.reshape(BLOC, D, OC, H)          # bl, d, j, h
        out[c * BL:(c + 1) * BL] = v4.transpose(3, 0, 2, 1).reshape(
            BL, OC, D)
    return out


def kernel(u_predict, b=None, **kw):
    u_predict = np.asarray(u_predict, dtype=np.float32)
    if b is None:
        b = np.zeros((IC, OC), np.float32)
    b = np.asarray(b, dtype=np.float32)
    in_maps, general_b = _prep_inputs(u_predict, b)
    nc = _get_program(general_b)

    if os.environ.get("BASS_KERNEL_SIM"):
        from concourse.bass_interp import CoreSim
        sim = CoreSim(nc, trace=False)
        for name, arr in in_maps[0].items():
            sim.tensor(name)[:] = arr
        sim.simulate(check_with_hw=False)
        v0 = np.array(sim.tensor("vout"))
        out = np.empty((B, OC, D), np.float32)
        v4 = v0.reshape(BLOC, D, OC, H)
        out[:BL] = v4.transpose(3, 0, 2, 1).reshape(BL, OC, D)
        return out  # NOTE: only core 0 valid in sim mode

    from concourse import bass_utils
    trace = bool(os.environ.get("BASS_KERNEL_TRACE"))
    res = bass_utils.run_bass_kernel_spmd(
        nc, in_maps, core_ids=list(range(NCORES)), trace=trace)
    kernel.last_results = res
    return _gather_output(res.results)


# revision 15
# speedup vs baseline: 2.1159x; 1.2526x over previous
"""AgreementRouting (CapsNet dynamic routing) Trainium2 kernel.

Data-parallel over batch B=128 across 8 cores (B_local=16 per core).

Per core, u lives in SBUF twice, as fp16:
  u16: partition p = b_loc*16 + d   (b_loc in [0,8), d in [0,16))
       free       = (j in [0,10), h in [0,2), i in [0,1152))
  uT:  partition  = i_lo = i % 128
       free       = (h, j, ci = i//128 in [0,9), p = (b_loc, d))
local batch index beta = h*8 + b_loc.

Structure per routing iteration:
  W1: PE accumulating matmuls a_c = sum_j sfat[j].T @ u16[j]    (fp16)
      (sfat = block-diagonal stationary holding the unnormalized s)
  bb += f80 * a_c                   (DVE STT, squash scale f
                                     folded into the logit update)
  softmax in i-major layout: PE-transpose bb 128-col chunks into
      bbT [i_lo, (ci, j, b)] f32 PSUM, exp on ACT, Z = sum over j via
      strided DVE reduce, c16 = e * recip(Z) with a stride-0 broadcast
      -> cT16 [i_lo, (ci, j, b)] fp16 directly in the transposed layout
  W4: weighted sum on PE with uT as the *stationary*:
      s_ps[(b,d), b'] += sum_i uT[i,(b,d)] * cT16[i,(j,b')]  (9 chunks)
      then a masked DVE reduce extracts the b'==b diagonal into s.
  squash scale f80 computed entirely in (j,b)-partition layout
      (PE matmul + masked STT), no SBUF-shuffle DMAs.

Numerics vs the fp32 oracle: absmax/scale ~ 5e-4.
"""

import os
import sys

import numpy as np

for _p in ("/opt/trn_rl_repo", "/opt/trn_rl_repo/concourse"):
    if _p not in sys.path and os.path.isdir(_p):
        sys.path.insert(0, _p)

B, IC, OC, D = 128, 1152, 10, 16
NCORES = 8
BL = B // NCORES          # 16 local batch
H = 2                     # halves of local batch
BLOC = BL // H            # 8
NI = IC                   # 1152
NC9 = NI // 128           # 9 i-chunks of 128
EPS = 1e-8
NITER = 3
CHUNKS = [(0, 512), (512, 1024), (1024, 1152)]
TIME_REPS = int(os.environ.get("K_TIME_REPS", "1"))  # whole-program reps

_PROG_CACHE = {}


def _build_consts():
    """Host-side constant selector/mask matrices."""
    # base8[(b,d), b2] = 1 if b==b2                      -> [128, 8] f32
    base8 = np.zeros((BLOC * D, BLOC), np.float32)
    for b in range(BLOC):
        base8[b * D:(b + 1) * D, b] = 1.0
    drep = np.ascontiguousarray(base8.T)
    # b82a[(b,d), (j,b2)] = 1 if b==b2                   -> [128, 80]
    b82a = np.tile(base8, (1, OC)).astype(np.float32)
    # jmask[(j,b), j2] = 1 if j==j2                      -> [80, 10] f32
    jmask = np.zeros((OC * BLOC, OC), np.float32)
    for j in range(OC):
        jmask[j * BLOC:(j + 1) * BLOC, j] = 1.0
    ident80 = np.eye(OC * BLOC, dtype=np.float32)
    return dict(base8=base8, drep=drep,
                b82a32=b82a, b82a16=b82a.astype(np.float16),
                jmask=jmask, ident80=ident80)


def _build_program(general_b):
    import concourse.bacc as bacc
    import concourse.mybir as mybir
    import concourse.tile as tile

    dt = mybir.dt
    AF = mybir.ActivationFunctionType
    ALU = mybir.AluOpType
    AX = mybir.AxisListType

    # Force a single shared ACT table (Exp+Ln+Copy+Identity in one set) so
    # the table-load pass emits one load instead of thrashing per func.
    from concourse import hw_specs as _hws
    _orig_tabs = _hws.get_activation_tables
    _keep = "natural_log_exp_and_others"

    def _patched_tabs(arch, __orig=_orig_tabs, __keep=_keep):
        tabs = __orig(arch)
        return {n: (s if n == __keep else set()) for n, s in tabs.items()}

    bacc.get_activation_tables = _patched_tabs

    nc = bacc.Bacc("TRN2", target_bir_lowering=False, debug=False)

    # ---- DRAM I/O ----
    u16_d = nc.dram_tensor("u16", [128, OC, H, NI], dt.float16,
                           kind="ExternalInput").ap()
    uT_d = nc.dram_tensor("uT", [128, H, OC, NC9 * 128], dt.float16,
                          kind="ExternalInput").ap()
    base8_d = nc.dram_tensor("base8", [BLOC * D, BLOC], dt.float32,
                             kind="ExternalInput").ap()
    drep_d = nc.dram_tensor("drep", [BLOC, BLOC * D], dt.float32,
                            kind="ExternalInput").ap()
    b82a32_d = nc.dram_tensor("b82a32", [BLOC * D, OC * BLOC], dt.float32,
                              kind="ExternalInput").ap()
    b82a16_d = nc.dram_tensor("b82a16", [BLOC * D, OC * BLOC], dt.float16,
                              kind="ExternalInput").ap()
    jmask_d = nc.dram_tensor("jmask", [OC * BLOC, OC], dt.float32,
                             kind="ExternalInput").ap()
    ident80_d = nc.dram_tensor("ident80", [OC * BLOC, OC * BLOC], dt.float32,
                               kind="ExternalInput").ap()
    if general_b:
        c0_d = nc.dram_tensor("c0rep", [128, OC, NI], dt.float16,
                              kind="ExternalInput").ap()
        bb0_d = nc.dram_tensor("bb0", [OC * BLOC, NI], dt.float32,
                               kind="ExternalInput").ap()
    out_d = nc.dram_tensor("vout", [128, 2 * OC], dt.float32,
                           kind="ExternalOutput").ap()

    # ---- static SBUF ----
    def sb(name, shape, dtype):
        return nc.alloc_sbuf_tensor(name, list(shape), dtype).ap()

    u16 = sb("u16_sb", [128, OC * H * NI], dt.float16)       # 46KB/part
    uT = sb("uT_sb", [128, H * OC * NC9 * 128], dt.float16)  # 46KB/part
    base8_sb = sb("base8_sb", [BLOC * D, BLOC], dt.float32)
    drep_sb = sb("drep_sb", [BLOC, BLOC * D], dt.float32)
    b82a32_sb = sb("b82a32_sb", [BLOC * D, OC * BLOC], dt.float32)
    b82a16_sb = sb("b82a16_sb", [BLOC * D, OC * BLOC], dt.float16)
    jmask_sb = sb("jmask_sb", [OC * BLOC, OC], dt.float32)
    ident80_sb = sb("ident80_sb", [OC * BLOC, OC * BLOC], dt.float32)
    bb = [sb(f"bbsb{h}", [OC * BLOC, NI], dt.float32) for h in range(H)]
    sfat = [[sb(f"sfat{j}_{h}", [128, OC * BLOC], dt.float16)
             for h in range(H)] for j in range(OC)]
    f80 = [sb(f"f80_{h}", [OC * BLOC, 1], dt.float32) for h in range(H)]
    s_sb = sb("s_sb", [128, H * OC], dt.float32)
    # mini-squash scratch (per h), all in [80, *] layout
    ssqh = [sb(f"ssqh{h}", [128, OC], dt.float16) for h in range(H)]
    jscr = [sb(f"jscr{h}", [OC * BLOC, OC], dt.float32) for h in range(H)]
    sq80 = [sb(f"sq80_{h}", [OC * BLOC, 1], dt.float32) for h in range(H)]
    seps80 = [sb(f"seps80_{h}", [OC * BLOC, 1], dt.float32) for h in range(H)]
    lnx80 = [sb(f"lnx80_{h}", [OC * BLOC, 1], dt.float32) for h in range(H)]
    r80 = [sb(f"r80_{h}", [OC * BLOC, 1], dt.float32) for h in range(H)]
    t180 = [sb(f"t180_{h}", [OC * BLOC, 1], dt.float32) for h in range(H)]
    den80 = [sb(f"den80_{h}", [OC * BLOC, 1], dt.float32) for h in range(H)]
    rec80 = [sb(f"rec80_{h}", [OC * BLOC, 1], dt.float32) for h in range(H)]
    # W4 extraction scratch
    mskd = [sb(f"mskd{h}", [128, OC * BLOC], dt.float32) for h in range(H)]
    # final squash scratch
    ssq_sb = sb("ssq_sb", [128, H * OC], dt.float32)
    seps_sb = sb("seps_sb", [BLOC, H * OC], dt.float32)
    lnx_sb = sb("lnx_sb", [BLOC, H * OC], dt.float32)
    r_sb = sb("r_sb", [BLOC, H * OC], dt.float32)
    t1_sb = sb("t1_sb", [BLOC, H * OC], dt.float32)
    den_sb = sb("den_sb", [BLOC, H * OC], dt.float32)
    rec_sb = sb("rec_sb", [BLOC, H * OC], dt.float32)
    f_sb = sb("f_sb", [BLOC, H * OC], dt.float32)
    v_sb = sb("v_sb", [128, H * OC], dt.float32)

    def uview(j, h):
        off = (j * H + h) * NI
        return u16[:, off:off + NI]

    def uTview(h, j, ci):
        off = ((h * OC + j) * NC9 + ci) * 128
        return uT[:, off:off + 128]

    with tile.TileContext(nc) as tc:
        from contextlib import ExitStack
        with ExitStack() as ctx:
            psA = ctx.enter_context(
                tc.tile_pool(name="psA", bufs=2, space="PSUM"))
            psB = ctx.enter_context(
                tc.tile_pool(name="psB", bufs=2, space="PSUM"))
            psS = ctx.enter_context(
                tc.tile_pool(name="psS", bufs=2, space="PSUM"))
            sc = ctx.enter_context(
                tc.tile_pool(name="sc", bufs=int(os.environ.get("K_SCBUFS", "3"))))
            ec = ctx.enter_context(
                tc.tile_pool(name="ec", bufs=int(os.environ.get("K_ECBUFS", "3"))))

            for _rep in range(TIME_REPS):
                # ---- loads: u16 first (gates init+W1), consts, then uT ----
                for j in range(OC):
                    for h in range(H):
                        off = (j * H + h) * NI
                        nc.sync.dma_start(
                            u16[:, off:off + NI], u16_d[:, j, h, :])
                nc.sync.dma_start(base8_sb[:], base8_d)
                nc.sync.dma_start(drep_sb[:], drep_d)
                nc.sync.dma_start(b82a32_sb[:], b82a32_d)
                nc.sync.dma_start(b82a16_sb[:], b82a16_d)
                nc.sync.dma_start(jmask_sb[:], jmask_d)
                nc.sync.dma_start(ident80_sb[:], ident80_d)
                for h in range(H):
                    for j in range(OC):
                        off = (h * OC + j) * NC9 * 128
                        nc.sync.dma_start(
                            uT[:, off:off + NC9 * 128], uT_d[:, h, j, :])

                # ---- init bb and sfat ----
                for h in range(H):
                    if general_b:
                        nc.sync.dma_start(bb[h][:], bb0_d)
                    else:
                        nc.gpsimd.memset(bb[h][:], 0.0)
                for j in range(OC):
                    for h in range(H):
                        nc.gpsimd.memset(sfat[j][h][:], 0.0)

                def build_sfat(j, h):
                    col = 2 * j + h
                    nc.gpsimd.tensor_scalar(
                        out=sfat[j][h][:, j * BLOC:(j + 1) * BLOC],
                        in0=base8_sb[:],
                        scalar1=s_sb[:, col:col + 1],
                        scalar2=None,
                        op0=ALU.mult)

                def mini_squash(h):
                    """f80[h] <- squash scale, computed in [80,*] layout."""
                    s_h = s_sb[:, h::2]  # [128, OC] strided view
                    nc.vector.tensor_tensor(ssqh[h][:], s_h, s_h, op=ALU.mult)
                    sq_ps = psB.tile([OC * BLOC, OC], dt.float32, tag="bank",
                                     name="sq80_ps")
                    nc.tensor.matmul(sq_ps[:], b82a16_sb[:], ssqh[h][:],
                                     start=True, stop=True)
                    nc.vector.scalar_tensor_tensor(
                        out=jscr[h][:], in0=sq_ps[:], scalar=1.0,
                        in1=jmask_sb[:], op0=ALU.mult, op1=ALU.mult,
                        accum_out=sq80[h][:])
                    nc.vector.tensor_scalar_add(seps80[h][:], sq80[h][:], EPS)
                    nc.scalar.activation(lnx80[h][:], seps80[h][:], AF.Ln)
                    nc.scalar.activation(r80[h][:], lnx80[h][:], AF.Exp,
                                         scale=0.5)
                    nc.vector.tensor_scalar_add(t180[h][:], sq80[h][:], 1.0)
                    nc.vector.tensor_tensor(den80[h][:], t180[h][:], r80[h][:],
                                            op=ALU.mult)
                    nc.vector.reciprocal(rec80[h][:], den80[h][:])
                    nc.vector.tensor_tensor(f80[h][:], sq80[h][:],
                                            rec80[h][:], op=ALU.mult)

                # ---- init s0 ----
                if general_b:
                    c0_sb = sc.tile([128, OC * NI], dt.float16, tag="c0",
                                    name="c0_sb", bufs=1)
                    nc.sync.dma_start(c0_sb[:], c0_d)
                    for j in range(OC):
                        for h in range(H):
                            col = 2 * j + h
                            scr = sc.tile([128, NI], dt.float16, tag="scr",
                                          name="scr")
                            nc.vector.scalar_tensor_tensor(
                                out=scr[:], in0=uview(j, h), scalar=1.0,
                                in1=c0_sb[:, j * NI:(j + 1) * NI],
                                op0=ALU.mult, op1=ALU.mult,
                                accum_out=s_sb[:, col:col + 1])
                else:
                    # split the 20 (j,h) reduction units across DVE/ACT/GpSimd
                    units = [(j, h) for j in range(OC) for h in range(H)]
                    for idx, (j, h) in enumerate(units):
                        col = 2 * j + h
                        eng = ("dve", "act", "gp", "dve", "act")[idx % 5]
                        if eng == "dve":
                            nc.vector.reduce_sum(
                                s_sb[:, col:col + 1], uview(j, h), axis=AX.X)
                        elif eng == "gp":
                            # max(u*1, u) == u; accum_out gives the i-sum
                            scr = sc.tile([128, NI], dt.float16, tag="scr",
                                          name="scr")
                            nc.gpsimd.scalar_tensor_tensor(
                                out=scr[:], in0=uview(j, h), scalar=1.0,
                                in1=uview(j, h), op0=ALU.mult, op1=ALU.max,
                                accum_out=s_sb[:, col:col + 1])
                        else:
                            scr = sc.tile([128, NI], dt.float16, tag="scr",
                                          name="scr")
                            nc.scalar.activation(
                                scr[:], uview(j, h), AF.Identity,
                                accum_out=s_sb[:, col:col + 1])
                if not general_b:
                    nc.vector.tensor_scalar_mul(s_sb[:], s_sb[:], 1.0 / OC)
                for j in range(OC):
                    for h in range(H):
                        build_sfat(j, h)
                for h in range(H):
                    mini_squash(h)

                # ---- routing iterations ----
                for it in range(NITER):
                    last = it == NITER - 1
                    # stage-major emission: each engine streams all 6 chunks
                    # (2h x 3c) back-to-back, so cross-engine latency amortizes.
                    hc = [(h, ci) for h in range(H) for ci in range(len(CHUNKS))]
                    a_t = {}
                    for (h, ci) in hc:
                        c0, c1 = CHUNKS[ci]
                        w = c1 - c0
                        a_c = psB.tile([OC * BLOC, 512], dt.float32,
                                       tag="bank", name="a_c")
                        for j in range(OC):
                            nc.tensor.matmul(
                                a_c[:, :w], sfat[j][h],
                                uview(j, h)[:, c0:c1],
                                start=(j == 0), stop=(j == OC - 1))
                        a_t[(h, ci)] = a_c
                    for (h, ci) in hc:
                        c0, c1 = CHUNKS[ci]
                        # GPSIMD cannot access PSUM (a_c), so DVE only here
                        nc.vector.scalar_tensor_tensor(
                            out=bb[h][:, c0:c1], in0=a_t[(h, ci)][:, :c1 - c0],
                            scalar=f80[h][:, 0:1], in1=bb[h][:, c0:c1],
                            op0=ALU.mult, op1=ALU.add)
                    # ---- softmax in i-major layout ----
                    # pieces: (lo: ci 0..4, hi: ci 5..8), each within a bank
                    PIECES = [(0, 5), (5, 9)]
                    hp = [(h, p) for h in range(H) for p in range(2)]
                    bbT_t = {}
                    for (h, p) in hp:
                        p0, p1 = PIECES[p]
                        w = (p1 - p0) * 80
                        bbT = psA.tile([128, w], dt.float32,
                                       tag=("lo", "hi")[p], name="bbT")
                        for ci in range(p0, p1):
                            nc.tensor.transpose(
                                bbT[:, (ci - p0) * 80:(ci - p0 + 1) * 80],
                                bb[h][:, ci * 128:(ci + 1) * 128],
                                ident80_sb[:])
                        bbT_t[(h, p)] = bbT
                    eT = {}
                    for h in range(H):
                        eT[h] = ec.tile([128, NC9 * 80], dt.float32,
                                        tag="eT", name="eT")
                    for (h, p) in hp:
                        p0, p1 = PIECES[p]
                        nc.scalar.activation(eT[h][:, p0 * 80:p1 * 80],
                                             bbT_t[(h, p)][:], AF.Exp)
                    zrT = {}
                    for h in range(H):
                        zrT[h] = ec.tile([128, NC9 * BLOC], dt.float32,
                                         tag="zrT", name="zrT")
                    for (h, p) in hp:
                        p0, p1 = PIECES[p]
                        ncc = p1 - p0
                        nc.vector.reduce_sum(
                            zrT[h][:, p0 * BLOC:p1 * BLOC].rearrange(
                                "p (ci b) -> p ci b", ci=ncc),
                            eT[h][:, p0 * 80:p1 * 80].rearrange(
                                "p (ci j b) -> p ci b j", ci=ncc, j=OC),
                            axis=AX.X)
                    for h in range(H):
                        nc.vector.reciprocal(zrT[h][:], zrT[h][:])
                    cT16 = {}
                    for h in range(H):
                        cT16[h] = ec.tile([128, NC9 * 80], dt.float16,
                                          tag="ct", name="cT16")
                    for (h, p) in hp:
                        p0, p1 = PIECES[p]
                        ncc = p1 - p0
                        eng = nc.gpsimd if h == 1 else nc.vector
                        eng.tensor_tensor(
                            cT16[h][:, p0 * 80:p1 * 80].rearrange(
                                "p (ci j b) -> p ci j b", ci=ncc, j=OC),
                            eT[h][:, p0 * 80:p1 * 80].rearrange(
                                "p (ci j b) -> p ci j b", ci=ncc, j=OC),
                            zrT[h][:, p0 * BLOC:p1 * BLOC].rearrange(
                                "p (ci b) -> p ci b", ci=ncc)[:, :, None, :]
                            .broadcast_to([128, ncc, OC, BLOC]),
                            op=ALU.mult)

                    # ---- W4: PE weighted-sum with uT stationary; masked
                    # reduce extracts the b'==b diagonal ----
                    for h in range(H):
                        s_ps = psS.tile([128, OC * BLOC], dt.float32,
                                        tag="sps", name="s_ps")
                        for j in range(OC):
                            for ci in range(NC9):
                                nc.tensor.matmul(
                                    s_ps[:, j * BLOC:(j + 1) * BLOC],
                                    uTview(h, j, ci),
                                    cT16[h][:, ci * 80 + j * BLOC:
                                            ci * 80 + (j + 1) * BLOC],
                                    start=(ci == 0), stop=(ci == NC9 - 1))
                        nc.vector.tensor_tensor(mskd[h][:], s_ps[:],
                                                b82a32_sb[:], op=ALU.mult)
                        nc.vector.reduce_sum(
                            s_sb[:, h::2],
                            mskd[h][:].rearrange("p (j b) -> p j b", j=OC),
                            axis=AX.X)
                        if not last:
                            for j in range(OC):
                                build_sfat(j, h)
                            mini_squash(h)

                # ---- final squash -> v ----
                nc.vector.tensor_tensor(ssq_sb[:], s_sb[:], s_sb[:], op=ALU.mult)
                sq_ps = psB.tile([BLOC, H * OC], dt.float32, tag="bank",
                                 name="sq_ps")
                nc.tensor.matmul(sq_ps[:], base8_sb[:], ssq_sb[:],
                                 start=True, stop=True)
                nc.vector.tensor_scalar_add(seps_sb[:], sq_ps[:], EPS)
                nc.scalar.activation(lnx_sb[:], seps_sb[:], AF.Ln)
                nc.scalar.activation(r_sb[:], lnx_sb[:], AF.Exp, scale=0.5)
                nc.vector.tensor_scalar_add(t1_sb[:], sq_ps[:], 1.0)
                nc.vector.tensor_tensor(den_sb[:], t1_sb[:], r_sb[:],
                                        op=ALU.mult)
                nc.vector.reciprocal(rec_sb[:], den_sb[:])
                nc.vector.tensor_tensor(f_sb[:], sq_ps[:], rec_sb[:],
                                        op=ALU.mult)
                f_ps = psB.tile([128, H * OC], dt.float32, tag="bank",
                                name="f_ps")
                nc.tensor.matmul(f_ps[:], drep_sb[:], f_sb[:],
                                 start=True, stop=True)
                nc.vector.tensor_tensor(v_sb[:], s_sb[:], f_ps[:], op=ALU.mult)
                nc.sync.dma_start(out_d, v_sb[:])

    nc.compile()
    return nc


def _get_program(general_b):
    key = bool(general_b)
    if key not in _PROG_CACHE:
        _PROG_CACHE[key] = _build_program(key)
    return _PROG_CACHE[key]


def _prep_inputs(u_predict, b):
    """Host-side shard + layout transform. Returns (in_maps, general_b)."""
    general_b = bool(np.any(b != 0.0))
    consts = _build_consts()
    u16 = u_predict.astype(np.float16)
    u6 = u16.reshape(NCORES, H, BLOC, IC, OC, D)
    ut = np.ascontiguousarray(u6.transpose(0, 2, 5, 4, 1, 3))
    ut = ut.reshape(NCORES, 128, OC, H, NI)
    # uT[c, i_lo, h, j, ci*128 + p] = ut[c, p, j, h, ci*128 + i_lo]
    u5 = ut.reshape(NCORES, 128, OC, H, NC9, 128)
    uTt = np.ascontiguousarray(u5.transpose(0, 5, 3, 2, 4, 1))
    uTt = uTt.reshape(NCORES, 128, H, OC, NC9 * 128)

    extra = {}
    if general_b:
        bm = b.astype(np.float64)
        e = np.exp(bm - bm.max(axis=1, keepdims=True))
        c0 = (e / e.sum(axis=1, keepdims=True)).astype(np.float16)  # [IC, OC]
        c0rep = np.ascontiguousarray(
            np.broadcast_to(c0.T[None, :, :], (128, OC, NI))).astype(
                np.float16)
        bt = b.astype(np.float32).T  # [OC, NI]
        bb0 = np.ascontiguousarray(
            np.repeat(bt[:, None, :], BLOC, axis=1)).reshape(OC * BLOC, NI)
        extra = {"c0rep": c0rep, "bb0": bb0}

    in_maps = []
    for c in range(NCORES):
        m = {"u16": ut[c], "uT": uTt[c]}
        m.update(consts)
        m.update(extra)
        in_maps.append(m)
    return in_maps, general_b


def _gather_output(results):
    out = np.empty((B, OC, D), np.float32)
    for c in range(NCORES):
        v = results[c]["vout"]                  # [p=(bl,d), col=(j*2+h)]
        v4 = v.reshape(BLOC, D, OC, H)          # bl, d, j, h
        out[c * BL:(c + 1) * BL] = v4.transpose(3, 0, 2, 1).reshape(
            BL, OC, D)
    return out


def kernel(u_predict, b=None, **kw):
    u_predict = np.asarray(u_predict, dtype=np.float32)
    if b is None:
        b = np.zeros((IC, OC), np.float32)
    b = np.asarray(b, dtype=np.float32)
    in_maps, general_b = _prep_inputs(u_predict, b)
    nc = _get_program(general_b)

    if os.environ.get("BASS_KERNEL_SIM"):
        from concourse.bass_interp import CoreSim
        sim = CoreSim(nc, trace=False)
        for name, arr in in_maps[0].items():
            sim.tensor(name)[:] = arr
        sim.simulate(check_with_hw=False)
        v0 = np.array(sim.tensor("vout"))
        out = np.empty((B, OC, D), np.float32)
        v4 = v0.reshape(BLOC, D, OC, H)
        out[:BL] = v4.transpose(3, 0, 2, 1).reshape(BL, OC, D)
        return out  # NOTE: only core 0 valid in sim mode

    from concourse import bass_utils
    trace = bool(os.environ.get("BASS_KERNEL_TRACE"))
    res = bass_utils.run_bass_kernel_spmd(
        nc, in_maps, core_ids=list(range(NCORES)), trace=trace)
    kernel.last_results = res
    return _gather_output(res.results)
